# revision 1
# baseline (speedup 1.0000x reference)
"""Trainium2 kernel for nn_BaseEncoderDecoder_28166395527595.

Strategy: pure data parallel over batch B=512 across 8 NeuronCores
(B_local=64 per core), with algebraic restructuring so the serial
recurrences only contain the irreducible [B,H]@[H,H] matmuls:

 - The encoder input projection is folded:  x_t @ W_ih_e.T
     = emb_t @ W_ih_e[:, :E].T + pos_t @ W_ih_e[:, E:].T
   and emb = one_hot @ W_emb.T + b_emb, so the whole per-step input
   term becomes one big precomputed matmul
     X_enc = one_hot_inputs @ (W_ih_e[:, :E] @ W_emb).T + bias_enc[s]
   where bias_enc[s] folds the position one-hot (a row of W_ih_e[:, E:])
   and all biases. Same folding for the decoder input projection.
 - Per step the encoder/decoder scans then only do
   tanh(X_t + state @ W_hh.T) (+ attention / output head for decoder).
"""

import numpy as np

B, S, V, E, H = 512, 256, 128, 64, 128
N_CORES = 8
B_LOC = B // N_CORES
EPS = 1e-20

_compiled = {}


def _build():
    import jax
    import jax.numpy as jnp
    from jax import lax

    def attend(dec, enc_states, mask):
        # dec: [b,H]; enc_states: [S,b,H]; mask: [b,S] bool
        scores = jnp.einsum('bh,sbh->bs', dec, enc_states)
        scores = jnp.where(mask, scores, jnp.float32(-1e9))
        w = jax.nn.softmax(scores, axis=1)
        ctx = jnp.einsum('bs,sbh->bh', w, enc_states)
        return dec + ctx

    def fwd(oh_in, oh_out, mask,
            W_enc_x, bias_enc, W_hh_e_T,
            W_dec_x, bias_dec, W_hh_d_T,
            W_e2d_T, b_e2d, W_out_T, b_out, first_pred):
        b = oh_in.shape[0]
        # [S, b, H] precomputed encoder input projections
        X_enc = jnp.einsum('bsv,vh->sbh', oh_in, W_enc_x) + bias_enc[:, None, :]
        maskT = mask.T  # [S, b]

        def enc_step(state, inp):
            x_t, m_t = inp
            nxt = jnp.tanh(x_t + state @ W_hh_e_T)
            state = jnp.where(m_t[:, None], nxt, state)
            return state, state

        state0 = jnp.zeros((b, H), dtype=jnp.float32)
        _, enc_states = lax.scan(enc_step, state0, (X_enc, maskT))  # [S,b,H]

        dec0 = enc_states[-1] @ W_e2d_T + b_e2d
        dec0 = attend(dec0, enc_states, mask)

        X_dec = jnp.einsum('bsv,vh->sbh', oh_out[:, :-1], W_dec_x) + bias_dec

        def dec_step(dec, x_t):
            nxt = jnp.tanh(x_t + dec @ W_hh_d_T)
            nxt = attend(nxt, enc_states, mask)
            pred = jax.nn.log_softmax(nxt @ W_out_T + b_out, axis=1)
            return nxt, pred

        _, preds = lax.scan(dec_step, dec0, X_dec)  # [S-1, b, V]
        out = jnp.concatenate([first_pred[None], preds], axis=0)  # [S,b,V]
        return jnp.transpose(out, (1, 0, 2))  # [b,S,V]

    in_axes = (0, 0, 0) + (None,) * 11
    return jax.pmap(fwd, in_axes=in_axes, out_axes=0)


def kernel(one_hot_inputs, one_hot_outputs, mask_inference_inputs,
           W_emb, b_emb, W_ih_e, W_hh_e, b_ih_e, b_hh_e,
           W_e2d, b_e2d, W_ih_d, W_hh_d, b_ih_d, b_hh_d, W_out, b_out):
    one_hot_inputs = np.asarray(one_hot_inputs, dtype=np.float32)
    one_hot_outputs = np.asarray(one_hot_outputs, dtype=np.float32)
    mask = np.asarray(mask_inference_inputs)
    f32 = lambda a: np.asarray(a, dtype=np.float32)
    W_emb, b_emb = f32(W_emb), f32(b_emb)
    W_ih_e, W_hh_e, b_ih_e, b_hh_e = map(f32, (W_ih_e, W_hh_e, b_ih_e, b_hh_e))
    W_e2d, b_e2d = f32(W_e2d), f32(b_e2d)
    W_ih_d, W_hh_d, b_ih_d, b_hh_d = map(f32, (W_ih_d, W_hh_d, b_ih_d, b_hh_d))
    W_out, b_out = f32(W_out), f32(b_out)

    # --- host-side parameter folding (tiny matrices) ---
    Wx_e = W_ih_e[:, :E]                     # [H, E]
    W_enc_x = (Wx_e @ W_emb).T               # [V, H]
    pos = np.eye(S, V, dtype=np.float32)     # [S, V]
    bias_enc = (pos @ W_ih_e[:, E:].T        # [S, H] position contribution
                + Wx_e @ b_emb + b_ih_e + b_hh_e)
    W_dec_x = (W_ih_d @ W_emb).T             # [V, H]
    bias_dec = W_ih_d @ b_emb + b_ih_d + b_hh_d  # [H]
    first_pred = np.log(np.full((B_LOC, V), EPS, dtype=np.float32))
    first_pred[:, 0] = 0.0

    if 'fn' not in _compiled:
        _compiled['fn'] = _build()
    fn = _compiled['fn']

    sh = lambda a: a.reshape((N_CORES, B_LOC) + a.shape[1:])
    out = fn(sh(one_hot_inputs), sh(one_hot_outputs), sh(mask),
             W_enc_x, bias_enc.astype(np.float32), W_hh_e.T.copy(),
             W_dec_x, bias_dec.astype(np.float32), W_hh_d.T.copy(),
             W_e2d.T.copy(), b_e2d, W_out.T.copy(), b_out, first_pred)
    return np.asarray(out).reshape(B, S, V)

